# revision 3
# baseline (speedup 1.0000x reference)
"""Trainium2 Bass kernel for char-CNN (embed lookup + conv1d(K=5,pad=2) + bias + maxpool).

Math: out[n, f] = max_w ( b[f] + sum_k sum_d  E[ids[n, w+k-2], d] * Wc[f, d, k] )

v3 strategy (fp16 hi + fp8 DoubleRow correction):
  * Host folds G[k][v, f] = sum_d E[v, d] * Wc[f, d, k]; lookup+conv becomes
    y[n,:,w] = sum_k G[k][ids[n,w+k-2], :] + b, via one-hot matmuls
    contracting over the vocab (96 rows + bias row 96).
  * ONE int16 one-hot value 0x3C01 serves two passes:
      - read as fp16 it is exactly 1.0009765625 = C; the hi tables store
        fp16(G/C), so the hi pass recovers G to ~2^-12 relative.
      - its low byte 0x01 is exactly 2^-9 in e4m3; the corr tables store
        e4m3((G - hi*C) * 2^9), recovering G to ~5.6e-5 absolute
        (measured end-to-end max rel err 3.1e-3 vs the 2e-2 budget).
  * The corr pass runs in fp8 DoubleRow mode pairing taps (k, k+1): plane 1
    of the one-hot holds the indicators shifted by one slot, so the two
    k-tiles of one matmul read taps k and k+1 at a 2560-byte pair stride
    (the ISA requires pair stride % 16 == 0).  5 hi + 3 corr matmuls per
    32-token half replace the baseline's 10 — PE work drops ~20% and the
    broadcast matmuls are gone entirely.
  * Corr matmuls accumulate into the hi PSUM directly (verified exact: the
    fp8 dot magnitudes are ~1e-2, below the fp8 path's precision knee).
  * ids reach the 96 compare partitions via a stride-0 broadcast DMA with
    i32 -> fp16 conversion; is_equal then runs in the DVE's 2x mode
    (all 2-byte operands).  Max-pool on DVE; PE runs only tap matmuls.
"""

import numpy as np
import ml_dtypes

import concourse.bass as bass
import concourse.bacc as bacc
import concourse.mybir as mybir
from concourse.tile import TileContext
from concourse.bass_utils import run_bass_kernel_spmd

# Problem shapes (hardcoded per contract)
N, W = 32768, 16
VOCAB, D, F, K = 96, 100, 100, 5
N_CORES = 8
NSH = N // N_CORES            # tokens per core = 4096
UNIT = 64                     # tokens per pipeline unit
NUNIT = NSH // UNIT           # 64
GROUP = 512                   # tokens per output DMA
NGROUP = NSH // GROUP         # 8
UPG = GROUP // UNIT           # units per group = 8
VP = 128                      # one-hot partitions: 96 vocab + bias row 96 + zero pad
WP = W + 4                    # padded char slots per token (pad 2 left, 2 right)
FP = 112                      # F padded so the DoubleRow weight k-tile stride % 16 == 0
NP = 3                        # corr tap-pairs: (0,1), (2,3), (4,-)
OHV16 = 15361                 # 0x3C01: fp16 C; low byte = e4m3 2^-9
C = 1.0009765625              # fp16 value of 0x3C01

f8 = mybir.dt.float8e4
f16 = mybir.dt.float16
f32 = mybir.dt.float32
i16 = mybir.dt.int16
i32 = mybir.dt.int32


def build_nc():
    nc = bacc.Bacc("TRN2", target_bir_lowering=False)

    ids_d = nc.dram_tensor("ids", [NSH, W], i32, kind="ExternalInput")
    ghi_d = nc.dram_tensor("ghi", [VP, K * F], f16, kind="ExternalInput")
    gco_d = nc.dram_tensor("gco", [VP, NP * 2 * FP], f8, kind="ExternalInput")
    iota_d = nc.dram_tensor("iota", [VOCAB, 1], f32, kind="ExternalInput")
    oinit_d = nc.dram_tensor("oinit", [VP, 2 * UNIT * WP], i16, kind="ExternalInput")
    out_d = nc.dram_tensor("out", [NGROUP, F, GROUP], f32, kind="ExternalOutput")

    with TileContext(nc) as tc:
        with (
            tc.tile_pool(name="consts", bufs=1) as consts,
            tc.tile_pool(name="outp", bufs=2) as outp,
            tc.tile_pool(name="bcp", bufs=4) as bcp,
            tc.tile_pool(name="psB", bufs=4, space="PSUM") as psB,
        ):
            iota_t = consts.tile([VOCAB, 1], f32)
            nc.sync.dma_start(out=iota_t, in_=iota_d[:, :])
            # touch DVE with the fused is_equal+mult opcode early
            dve_warm = consts.tile([VOCAB, 1], f32, tag="dve_warm")
            nc.vector.tensor_scalar(
                out=dve_warm[:, :],
                in0=iota_t[:, :],
                scalar1=iota_t[:, 0:1],
                scalar2=float(OHV16),
                op0=mybir.AluOpType.is_equal,
                op1=mybir.AluOpType.mult,
            )

            ghi = consts.tile([VP, K, F], f16)
            nc.sync.dma_start(out=ghi.rearrange("v k f -> v (k f)"), in_=ghi_d[:, :])
            gco = consts.tile([VP, NP, 2, FP], f8)
            nc.sync.dma_start(
                out=gco.rearrange("v p j f -> v (p j f)"), in_=gco_d[:, :]
            )

            # Four persistent one-hot tiles, int16 [VP, 2 planes, UNIT, W+4]
            # (t-major). Plane 0: char w at slot w+2; plane 1: shifted left by
            # one (char w at slot w+1) so DoubleRow k-tile pairs read taps
            # (k, k+1). Zero background; row 96 = OHV16 (bias row).
            o_tiles = []
            for j in range(4):
                ot = consts.tile([VP, 2, UNIT, WP], i16, tag=f"onehot{j}")
                nc.sync.dma_start(
                    out=ot.rearrange("v p t w -> v (p t w)"),
                    in_=oinit_d[:, :],
                )
                o_tiles.append(ot)

            def onehot(u):
                # ids -> 96 partitions via broadcast DMA (i32 -> f16), then
                # fused compare*OHV16 into the two int16 one-hot planes.
                bc = bcp.tile([VOCAB, UNIT, W], f16, tag="bc")
                src = ids_d[u * UNIT : (u + 1) * UNIT, :].rearrange("t w -> (t w)")
                nc.gpsimd.dma_start(
                    out=bc.rearrange("v t w -> v (t w)"),
                    in_=src.unsqueeze(0).to_broadcast([VOCAB, UNIT * W]),
                )
                o_t = o_tiles[u % 4]
                for pl, s0 in ((0, 2), (1, 1)):
                    nc.vector.tensor_scalar(
                        out=o_t[0:VOCAB, pl, :, s0 : s0 + W],
                        in0=bc[:, :, :],
                        scalar1=iota_t[:, 0:1],
                        scalar2=float(OHV16),
                        op0=mybir.AluOpType.is_equal,
                        op1=mybir.AluOpType.mult,
                    )

            # PE warmup: tiny matmuls keep the HAM activity window busy while
            # the init DMAs land, so real matmuls start at full clock.
            warm = psB.tile([1, 1], f32, tag="y0", name="warm")
            for _ in range(48):
                nc.tensor.matmul(
                    warm[0:1, 0:1],
                    iota_t[0:1, 0:1],
                    iota_t[0:1, 0:1],
                    start=True,
                    stop=True,
                    skip_group_check=True,
                )

            for u in range(min(4, NUNIT)):
                onehot(u)
            out_sb = None
            for u in range(NUNIT):
                g, uu = divmod(u, UPG)
                if uu == 0:
                    out_sb = outp.tile([F, GROUP], f32, tag="osb")

                o_t = o_tiles[u % 4]
                o16 = o_t.bitcast(f16)
                o8 = o_t.bitcast(f8).rearrange("v p t (w b) -> v p b t w", b=2)
                ys = [
                    psB.tile([FP, 32, W], f32, tag=f"y{h}", name=f"y{h}")
                    for h in range(2)
                ]
                for h in range(2):
                    t0, t1 = h * 32, (h + 1) * 32
                    for k in range(K):
                        nc.tensor.matmul(
                            ys[h][0:F, :, :],
                            ghi[:, k, :],
                            o16[:, 0, t0:t1, k : k + W],
                            start=(k == 0),
                            stop=False,
                            skip_group_check=True,
                        )
                    for p in range(NP):
                        nc.tensor.matmul(
                            ys[h][:, :, :],
                            gco[:, p, :, :],
                            o8[:, :, 0, t0:t1, 2 * p : 2 * p + W],
                            start=False,
                            stop=(p == NP - 1),
                            perf_mode=mybir.MatmulPerfMode.DoubleRow,
                            skip_group_check=True,
                        )

                # refill this unit's one-hot tile for unit u+4 (safe only
                # after the taps above have consumed it; WAR dep enforced)
                if u + 4 < NUNIT:
                    onehot(u + 4)

                # max over the 16 char positions
                for h in range(2):
                    nc.vector.reduce_max(
                        out=out_sb[:, uu * UNIT + h * 32 : uu * UNIT + (h + 1) * 32],
                        in_=ys[h][0:F, :, :],
                        axis=mybir.AxisListType.X,
                    )

                if uu == UPG - 1:
                    # stream this group's result out to DRAM (contiguous block)
                    nc.sync.dma_start(out=out_d[g, :, :], in_=out_sb[:, :])

    nc.compile()
    return nc


def make_consts(embed_table, conv_w, conv_b):
    # G[k][v, f] = sum_d E[v, d] * Wc[f, d, k] in float64
    G = np.einsum(
        "vd,fdk->kvf", embed_table.astype(np.float64), conv_w.astype(np.float64)
    )
    Gf = np.zeros((K, VP, F), np.float64)
    Gf[:, 0:VOCAB, :] = G
    Gf[2, VOCAB, :] = conv_b.astype(np.float64)  # bias rides center tap row 96

    hi_t = (Gf / C).astype(np.float32).astype(np.float16)            # [K, VP, F]
    resid = Gf - hi_t.astype(np.float64) * C
    co_t = (resid * 2.0**9).astype(np.float32).astype(ml_dtypes.float8_e4m3)

    ghi = np.ascontiguousarray(np.transpose(hi_t, (1, 0, 2))).reshape(VP, K * F)
    gco = np.zeros((VP, NP, 2, FP), dtype=ml_dtypes.float8_e4m3)
    for p in range(NP):
        for j in range(2):
            k = 2 * p + j
            if k < K:
                gco[:, p, j, 0:F] = co_t[k]
    gco = gco.reshape(VP, NP * 2 * FP)

    iota = np.arange(VOCAB, dtype=np.float32).reshape(VOCAB, 1)
    oinit = np.zeros((VP, 2 * UNIT * WP), np.int16)
    oinit[VOCAB, :] = OHV16
    return ghi, gco, iota, oinit


_NC_CACHE = {}

# Test-harness knobs (ignored by normal kernel() use)
TRACE = False
LAST_RESULT = None


def kernel(char_ids, embed_table, conv_w, conv_b):
    global LAST_RESULT
    char_ids = np.asarray(char_ids)
    ghi, gco, iota, oinit = make_consts(
        np.asarray(embed_table), np.asarray(conv_w), np.asarray(conv_b)
    )

    if "nc" not in _NC_CACHE:
        _NC_CACHE["nc"] = build_nc()
    nc = _NC_CACHE["nc"]

    in_maps = []
    for c in range(N_CORES):
        shard = np.ascontiguousarray(char_ids[c * NSH : (c + 1) * NSH])
        in_maps.append({"ids": shard, "ghi": ghi, "gco": gco, "iota": iota, "oinit": oinit})

    kwargs = {}
    if TRACE:
        kwargs = dict(trace=True, trace_cores=list(range(N_CORES)))
    res = run_bass_kernel_spmd(nc, in_maps, core_ids=list(range(N_CORES)), **kwargs)
    LAST_RESULT = res

    out = np.empty((N, F), np.float32)
    for c in range(N_CORES):
        o = res.results[c]["out"]  # [NGROUP, F, GROUP]
        out[c * NSH : (c + 1) * NSH] = o.transpose(0, 2, 1).reshape(NSH, F)
    return out


if __name__ == "__main__":
    import sys, tempfile

    nc = build_nc()
    print("build OK")
    if "--walrus" in sys.argv:
        from concourse.bass_utils import compile_bir_kernel

        with tempfile.TemporaryDirectory() as td:
            neff = compile_bir_kernel(nc.to_json_bytes(), td)
            print("WALRUS OK:", neff)
